# revision 8
# baseline (speedup 1.0000x reference)
"""Devign GGNN model on 8 Trainium2 NeuronCores.

Strategy (data-parallel over graphs, 4 graphs/core):
- Edge gather + scatter-add replaced by dense per-(graph, edge-type)
  adjacency matmuls: a = sum_t A_t @ (h @ W_t.T). A_t is built host-side
  from the integer edge lists (small exact counts, fp8-e4m3).
- Nodes packed (4x513 = 2052 rows, padded to 17x128 = 2176); each graph's
  adjacency strip touches exactly 5 source chunks (513*g starts at chunk 4g),
  so block-diagonality costs no extra matmuls and no per-graph padding.
- Nearly all matmuls run fp8-e4m3 DoubleRow (2 contraction rows/cycle):
  adjacency (A exact in fp8), messages (steps 1-3; step 0 is bf16 exact),
  GRU gates (fp8 copies of a and h as moving operands), and the conv head
  stage-1/2 (fp8 activations). The bf16 state h is kept for the GRU
  update path, so fp8 only touches matmul operands.
- All state is SBUF-resident in "transposed" layouts (feature dim on
  partitions): zero on-device transposes. GRU gate matmuls accumulate
  W_ih@a and W_hh@h in one PSUM group (ir+hr fused); conv head consumes
  packed fp8 [h | feat] planes directly as the channel dimension.
- Elementwise work is spread across Scalar(ACT)/Vector(DVE)/GpSimd so the
  PE stays the only bottleneck; GRU DVE intermediates are bf16 (2x DVE).
"""

import os
import sys

for _p in ("/opt/trn_rl_repo",):
    if os.path.isdir(_p) and _p not in sys.path:
        sys.path.append(_p)

import numpy as np
import ml_dtypes

BF16 = ml_dtypes.bfloat16
F8 = ml_dtypes.float8_e4m3

B, NN, IN, OUT, T, STEPS = 32, 513, 128, 256, 4, 4
CAT = OUT + IN
NCORES = 8
GPC = B // NCORES          # graphs per core = 4
NV = GPC * NN              # valid packed rows per core = 2052
KCH = 17                   # packed row chunks (2176 = 17 x 128)
NP = KCH * 128             # padded packed rows = 2176
SKC = 5                    # src chunks per graph strip (graph g: chunks 4g..4g+4)
SC = SKC * T               # strip chunk count incl types = 20
SL = [(0, 512), (512, 1024), (1024, 1536), (1536, 2048), (2048, NV)]
ASL = [(0, 320), (320, NN)]  # adjacency dst sub-slabs per graph (513 cols)
L1, P1 = NN - 2, 255       # conv1 out len, pool1 out len
L2Y, P2 = P1, 127          # conv2(k=1) len, final pooled len
L2Z = P1 - 1               # convc2(k=2) out len = 254

_prog_cache = {}


def _build_program(flags):
    import concourse.bacc as bacc
    import concourse.mybir as mybir
    import concourse.tile as tile

    has_bmsg, has_gru_b, has_conv_b, has_mlp_b = flags
    f32 = mybir.dt.float32
    bf16 = mybir.dt.bfloat16
    fp8 = mybir.dt.float8e4
    AF = mybir.ActivationFunctionType
    OP = mybir.AluOpType
    DR = mybir.MatmulPerfMode.DoubleRow

    nc = bacc.Bacc("TRN2", target_bir_lowering=False, debug=False,
                   enable_asserts=False, num_devices=NCORES)

    # ---- DRAM I/O (all pre-laid-out host side, partition dim first) ----
    d_feat = nc.dram_tensor("feat", [128, NP], bf16, kind="ExternalInput").ap()
    d_feat8 = nc.dram_tensor("feat8", [128, NP], fp8, kind="ExternalInput").ap()
    d_A = nc.dram_tensor("adj", [128, GPC, SC, NN], fp8, kind="ExternalInput").ap()
    d_wmsg = nc.dram_tensor("wmsg", [128, T * OUT], bf16, kind="ExternalInput").ap()
    d_wmsg8 = nc.dram_tensor("wmsg8", [128, 2, T * OUT], fp8, kind="ExternalInput").ap()
    d_wih8 = nc.dram_tensor("wih8", [128, 2, 3 * OUT], fp8, kind="ExternalInput").ap()
    d_whh8 = nc.dram_tensor("whh8", [128, 2, 3 * OUT], fp8, kind="ExternalInput").ap()
    d_whh0 = nc.dram_tensor("whh0", [128, 3 * OUT], bf16, kind="ExternalInput").ap()
    d_c1w = nc.dram_tensor("c1w", [128, 3, 2, 2, 128], fp8, kind="ExternalInput").ap()
    d_c2w = nc.dram_tensor("c2w", [128, 2, 2, 128], fp8, kind="ExternalInput").ap()
    d_cc1w = nc.dram_tensor("cc1w", [128, 3, 3, 3, 128], fp8, kind="ExternalInput").ap()
    d_cc2w = nc.dram_tensor("cc2w", [128, 3, 3, 2, 128], fp8, kind="ExternalInput").ap()
    d_mlpy = nc.dram_tensor("mlpy", [128, 2], bf16, kind="ExternalInput").ap()
    d_mlpz = nc.dram_tensor("mlpz", [128, 3], bf16, kind="ExternalInput").ap()
    if has_bmsg:
        d_bmsg = nc.dram_tensor("bmsg", [T, OUT], f32, kind="ExternalInput").ap()
        d_indeg = nc.dram_tensor("indeg", [T, NP], f32, kind="ExternalInput").ap()
    if has_gru_b:
        d_gbias = nc.dram_tensor("gbias", [128, 12], f32, kind="ExternalInput").ap()
    if has_conv_b:
        d_cbias = nc.dram_tensor("cbias", [128, 10], f32, kind="ExternalInput").ap()
    if has_mlp_b:
        d_mbias = nc.dram_tensor("mbias", [1, 2], f32, kind="ExternalInput").ap()
    d_out = nc.dram_tensor("out", [GPC], f32, kind="ExternalOutput").ap()

    def mm_acc(nct, ps, pairs):
        n = len(pairs)
        for i, (l, r) in enumerate(pairs):
            nct.tensor.matmul(ps, l, r, start=(i == 0), stop=(i == n - 1))

    with tile.TileContext(nc) as tc:
        from contextlib import ExitStack
        with ExitStack() as ctx:
            cpool = ctx.enter_context(tc.tile_pool(name="const", bufs=1))
            hpool = ctx.enter_context(tc.tile_pool(name="hstate", bufs=1))
            ypool = ctx.enter_context(tc.tile_pool(name="yact", bufs=2))
            zpool = ctx.enter_context(tc.tile_pool(name="zact", bufs=2))
            ps_hw = ctx.enter_context(
                tc.tile_pool(name="pshw", bufs=3, space="PSUM"))

            # ---- persistent tiles ----
            feat = cpool.tile([128, NP], bf16, tag="feat")
            wmsg = cpool.tile([128, T * OUT], bf16, tag="wmsg")
            wmsg8 = cpool.tile([128, 2, T * OUT], fp8, tag="wmsg8")
            wih8 = cpool.tile([128, 2, 3 * OUT], fp8, tag="wih8")
            whh8 = cpool.tile([128, 2, 3 * OUT], fp8, tag="whh8")
            whh0 = cpool.tile([128, 3 * OUT], bf16, tag="whh0")
            c1w = cpool.tile([128, 3, 2, 2, 128], fp8, tag="c1w")
            c2w = cpool.tile([128, 2, 2, 128], fp8, tag="c2w")
            cc1w = cpool.tile([128, 3, 3, 3, 128], fp8, tag="cc1w")
            cc2w = cpool.tile([128, 3, 3, 2, 128], fp8, tag="cc2w")
            mlpy = cpool.tile([128, 2], bf16, tag="mlpy")
            mlpz = cpool.tile([128, 3], bf16, tag="mlpz")
            hT = [hpool.tile([128, NP], bf16, tag=f"hT{m}", name=f"hT{m}")
                  for m in range(2)]
            h8 = hpool.tile([128, 2, NP], fp8, tag="h8")
            a8 = hpool.tile([128, 2, NP], fp8, tag="a8")
            # conv-head fp8 input planes: [h_m0 | h_m1 | feat]
            xpack = hpool.tile([128, 3, NP], fp8, tag="xpack")
            out_sb = cpool.tile([1, GPC], f32, tag="outsb")

            nc.sync.dma_start(out=wmsg[:], in_=d_wmsg[:])
            nc.sync.dma_start(out=wmsg8[:], in_=d_wmsg8[:])
            nc.sync.dma_start(out=feat[:, :1024], in_=d_feat[:, :1024])

            # HAM warmup: dummy matmuls spanning the input-DMA wait keep the
            # PE activity monitor from holding the 1.2 GHz throttle when the
            # first real matmuls issue (~12us in). Results are never read.
            warm = cpool.tile([128, 512], bf16, tag="warm")
            nc.vector.memset(warm[:], 0.0)
            for _wi in range(10):
                wps = ps_hw.tile([128, 512], f32, tag="pshw", name="warm")
                nc.tensor.matmul(wps[:], warm[:, :128], warm[:],
                                 start=True, stop=True)

            if has_conv_b:
                cbias = cpool.tile([128, 10], f32, tag="cbias")
                nc.sync.dma_start(out=cbias[:], in_=d_cbias[:])
            if has_mlp_b:
                mbias = cpool.tile([1, 2], f32, tag="mbias")
                nc.sync.dma_start(out=mbias[:], in_=d_mbias[:])

            # h0 = [feature | 0] is consumed in-place at step 0 (no copy);
            # hT tiles are first written by the step-0 GRU update.

            # ================= GGNN =================
            with ExitStack() as gctx:
                apool = gctx.enter_context(tc.tile_pool(name="adj", bufs=1))
                hwpool = gctx.enter_context(tc.tile_pool(name="hw", bufs=2))
                grupool = gctx.enter_context(tc.tile_pool(name="gru", bufs=3))
                ps_g = gctx.enter_context(
                    tc.tile_pool(name="psg", bufs=5, space="PSUM"))

                A_sb = apool.tile([128, GPC, SC, NN], fp8, tag="A")
                # g0's adjacency jumps the queue ahead of the second feat
                # half: the first hw chunks only need feat cols 0-1023, and
                # the step-0 A(g0) group is the first DMA-arrival stall
                nc.sync.dma_start(out=A_sb[:, 0], in_=d_A[:, 0])
                nc.sync.dma_start(out=feat[:, 1024:], in_=d_feat[:, 1024:])
                for g in range(1, GPC):
                    nc.gpsimd.dma_start(out=A_sb[:, g], in_=d_A[:, g])
                nc.sync.dma_start(out=wih8[:], in_=d_wih8[:])
                nc.sync.dma_start(out=whh8[:], in_=d_whh8[:])
                nc.sync.dma_start(out=whh0[:], in_=d_whh0[:])
                nc.gpsimd.dma_start(out=c1w[:], in_=d_c1w[:])
                nc.gpsimd.dma_start(out=c2w[:], in_=d_c2w[:])
                nc.gpsimd.dma_start(out=cc1w[:], in_=d_cc1w[:])
                nc.gpsimd.dma_start(out=cc2w[:], in_=d_cc2w[:])
                nc.gpsimd.dma_start(out=mlpy[:], in_=d_mlpy[:])
                nc.gpsimd.dma_start(out=mlpz[:], in_=d_mlpz[:])
                nc.gpsimd.dma_start(out=xpack[:, 2, :], in_=d_feat8[:])
                # adjacency writes only valid dst cols; zero the pad once.
                # h8 pad is read by chunk-16 message matmuls at s>=1 but the
                # GRU only writes the 4 valid cols of the last slab.
                nc.vector.memset(a8[:, :, NV:], 0.0)
                nc.vector.memset(h8[:, :, NV:], 0.0)

                if has_bmsg:
                    bmsg = cpool.tile([T, OUT], f32, tag="bmsg")
                    indeg = cpool.tile([T, NP], f32, tag="indeg")
                    nc.sync.dma_start(out=bmsg[:], in_=d_bmsg[:])
                    nc.sync.dma_start(out=indeg[:], in_=d_indeg[:])
                    bias_a = [cpool.tile([128, NP], bf16, tag=f"biasa{m}",
                                         name=f"biasa{m}") for m in range(2)]
                    for m in range(2):
                        for (s0, s1) in SL:
                            ps = ps_g.tile([128, s1 - s0], f32, tag="psg",
                                           name="psb")
                            nc.tensor.matmul(
                                ps[:], bmsg[:, m * 128:(m + 1) * 128],
                                indeg[:, s0:s1], start=True, stop=True)
                            nc.vector.tensor_copy(
                                out=bias_a[m][:, s0:s1], in_=ps[:])
                if has_gru_b:
                    gbias = cpool.tile([128, 12], f32, tag="gbias")
                    nc.sync.dma_start(out=gbias[:], in_=d_gbias[:])
                    bias_rz = cpool.tile([128, 4], f32, tag="biasrz")
                    nc.vector.tensor_add(
                        out=bias_rz[:], in0=gbias[:, 0:4], in1=gbias[:, 6:10])
                    nc.vector.tensor_scalar_mul(
                        out=bias_rz[:], in0=bias_rz[:], scalar1=0.125)

                for s in range(STEPS):
                    # --- messages: all 17 packed chunks once; step 0 is
                    # bf16-exact (128-dim input), steps 1-3 fp8 DoubleRow.
                    hw = hwpool.tile([128, KCH * T, 256], fp8, tag="hw")
                    for rc in range(KCH):
                        for tp in range(2):
                            ps = ps_hw.tile([128, 512], f32, tag="pshw")
                            if s == 0:
                                nc.tensor.matmul(
                                    ps[:], feat[:, rc * 128:(rc + 1) * 128],
                                    wmsg[:, tp * 512:(tp + 1) * 512],
                                    start=True, stop=True)
                            else:
                                nc.tensor.matmul(
                                    ps[:], h8[:, :, rc * 128:(rc + 1) * 128],
                                    wmsg8[:, :, tp * 512:(tp + 1) * 512],
                                    start=True, stop=True, perf_mode=DR)
                            h0 = rc * T + 2 * tp
                            dsc = 1.0 if s == 0 else 0.125
                            nc.scalar.activation(
                                hw[:, h0, :], ps[:, 0:256], AF.Copy, scale=dsc)
                            nc.vector.tensor_scalar_mul(
                                out=hw[:, h0 + 1, :], in0=ps[:, 256:512],
                                scalar1=dsc)

                    # --- adjacency matmul per graph strip (fp8 DoubleRow) ---
                    def a_phase(g):
                        base = g * NN
                        for m in range(2):
                            pa = [ps_g.tile([128, n1 - n0], f32, tag="psg",
                                            name=f"pa{n0}")
                                  for (n0, n1) in ASL]
                            for ps, (n0, n1) in zip(pa, ASL):
                                for i2 in range(SC // 2):
                                    nc.tensor.matmul(
                                        ps[:],
                                        hw[:, 16 * g + 2 * i2:
                                           16 * g + 2 * i2 + 2,
                                           m * 128:(m + 1) * 128],
                                        A_sb[:, g, 2 * i2: 2 * i2 + 2, n0:n1],
                                        start=(i2 == 0), stop=(i2 == SC // 2 - 1),
                                        perf_mode=DR)
                            for si, (ps, (n0, n1)) in enumerate(zip(pa, ASL)):
                                dst = a8[:, m, base + n0:base + n1]
                                if has_bmsg:
                                    nc.vector.tensor_add(
                                        out=dst, in0=ps[:],
                                        in1=bias_a[m][:, base + n0:base + n1])
                                elif si == 0:
                                    nc.scalar.copy(out=dst, in_=ps[:])
                                else:
                                    nc.vector.tensor_copy(out=dst, in_=ps[:])

                    # --- GRU, per row slab ---
                    def gru_slab(s0, s1):
                        w = s1 - s0
                        cs = slice(s0, s1)
                        rz = grupool.tile([128, 4, 512], bf16, tag="rz",
                                          name="rz")[:, :, :w]
                        nt = grupool.tile([128, 2, 512], bf16, tag="nt",
                                          name="nt")[:, :, :w]
                        for gc in range(4):
                            # one PSUM group accumulates ir+hr (iz+hz)
                            ps = ps_g.tile([128, 512], f32, tag="psg",
                                           name="psgr")[:, :w]
                            nc.tensor.matmul(
                                ps[:], wih8[:, :, gc * 128:(gc + 1) * 128],
                                a8[:, :, cs], start=True, stop=False,
                                perf_mode=DR)
                            if s == 0:
                                nc.tensor.matmul(
                                    ps[:], whh0[:, gc * 128:(gc + 1) * 128],
                                    feat[:, cs], start=False, stop=True)
                            else:
                                nc.tensor.matmul(
                                    ps[:], whh8[:, :, gc * 128:(gc + 1) * 128],
                                    h8[:, :, cs], start=False, stop=True,
                                    perf_mode=DR)
                            nc.scalar.activation(
                                rz[:, gc, :], ps[:], AF.Sigmoid, scale=0.125,
                                bias=bias_rz[:, gc:gc + 1] if has_gru_b else 0.0)
                        for j in range(2):
                            gc = 4 + j
                            pi = ps_g.tile([128, 512], f32, tag="psg",
                                           name="pgi")[:, :w]
                            nc.tensor.matmul(
                                pi[:], wih8[:, :, gc * 128:(gc + 1) * 128],
                                a8[:, :, cs], start=True, stop=True,
                                perf_mode=DR)
                            ph = ps_g.tile([128, 512], f32, tag="psg",
                                           name="pgh")[:, :w]
                            if s == 0:
                                nc.tensor.matmul(
                                    ph[:], whh0[:, gc * 128:(gc + 1) * 128],
                                    feat[:, cs], start=True, stop=True)
                            else:
                                nc.tensor.matmul(
                                    ph[:], whh8[:, :, gc * 128:(gc + 1) * 128],
                                    h8[:, :, cs], start=True, stop=True,
                                    perf_mode=DR)
                            if has_gru_b:
                                nc.vector.tensor_scalar_add(
                                    out=pi[:], in0=pi[:],
                                    scalar1=gbias[:, gc:gc + 1])
                                nc.vector.tensor_scalar_add(
                                    out=ph[:], in0=ph[:],
                                    scalar1=gbias[:, 6 + gc:7 + gc])
                            rhn = grupool.tile([128, 512], bf16, tag="rhn",
                                               name="rhn")[:, :w]
                            nc.vector.tensor_tensor(
                                out=rhn[:], in0=rz[:, j, :], in1=ph[:],
                                op=OP.mult)
                            nc.vector.tensor_add(out=pi[:], in0=pi[:], in1=rhn[:])
                            nc.scalar.activation(nt[:, j, :], pi[:], AF.Tanh,
                                                 scale=0.125)
                        for m in range(2):
                            d = grupool.tile([128, 512], bf16, tag="d",
                                             name="d")[:, :w]
                            if s == 0 and m == 1:
                                # h=0: h' = n - z*n
                                nc.vector.tensor_tensor(
                                    out=d[:], in0=rz[:, 3, :], in1=nt[:, 1, :],
                                    op=OP.mult)
                                nc.vector.tensor_sub(
                                    out=hT[1][:, cs], in0=nt[:, 1, :], in1=d[:])
                            else:
                                hsrc = feat if s == 0 else hT[m]
                                nc.vector.tensor_sub(
                                    out=d[:], in0=hsrc[:, cs], in1=nt[:, m, :])
                                nc.vector.tensor_tensor(
                                    out=d[:], in0=rz[:, 2 + m, :], in1=d[:],
                                    op=OP.mult)
                                nc.vector.tensor_add(
                                    out=hT[m][:, cs], in0=nt[:, m, :], in1=d[:])
                            # fp8 copy for the next consumer (messages+GRU
                            # rhs at s+1, conv-head planes after step 3)
                            if s < STEPS - 1:
                                nc.scalar.copy(
                                    out=h8[:, m, cs], in_=hT[m][:, cs])
                            else:
                                nc.scalar.copy(
                                    out=xpack[:, m, cs], in_=hT[m][:, cs])

                    # interleave: GRU slab k only needs graphs <= k+1,
                    # so its DVE/ACT tail overlaps later graphs' adjacency mms
                    a_phase(0)
                    a_phase(1)
                    gru_slab(*SL[0])
                    a_phase(2)
                    gru_slab(*SL[1])
                    a_phase(3)
                    gru_slab(*SL[2])
                    gru_slab(*SL[3])
                    gru_slab(*SL[4])

            # ================= conv heads =================
            # stage 1 + pool1 per graph (fp8 DoubleRow over packed planes),
            # then stage 2 batched over graph pairs, then per-graph heads.
            y1pA = hpool.tile([128, 2, GPC, P1], fp8, tag="y1pA")
            z1pA = hpool.tile([128, 3, GPC, P1], fp8, tag="z1pA")
            for g in range(GPC):
                base = g * NN
                y1 = ypool.tile([128, 2, L1], bf16, tag="y1")
                z1 = zpool.tile([128, 3, L1], bf16, tag="z1")
                for co in range(2):
                    ps = ps_hw.tile([128, L1], f32, tag="pshw")
                    for k in range(3):
                        nc.tensor.matmul(
                            ps[:], c1w[:, k, co, :, :],
                            xpack[:, 0:2, base + k: base + k + L1],
                            start=(k == 0), stop=(k == 2), perf_mode=DR)
                    nc.scalar.activation(
                        y1[:, co, :], ps[:], AF.Relu, scale=0.125,
                        bias=cbias[:, co:co + 1] if has_conv_b else 0.0)
                for co in range(3):
                    ps = ps_hw.tile([128, L1], f32, tag="pshw")
                    for k in range(3):
                        nc.tensor.matmul(
                            ps[:], cc1w[:, k, co, 0:2, :],
                            xpack[:, 0:2, base + k: base + k + L1],
                            start=(k == 0), stop=False, perf_mode=DR)
                    for k in range(3):
                        nc.tensor.matmul(
                            ps[:], cc1w[:, k, co, 2, :],
                            xpack[:, 2, base + k: base + k + L1],
                            start=False, stop=(k == 2))
                    nc.scalar.activation(
                        z1[:, co, :], ps[:], AF.Relu, scale=0.125,
                        bias=cbias[:, 4 + co:5 + co] if has_conv_b else 0.0)
                # pool1 (k=3, s=2) -> fp8 batched tiles
                y1t = ypool.tile([128, 2, P1], bf16, tag="y1t")
                z1t = zpool.tile([128, 3, P1], bf16, tag="z1t")
                for co in range(2):
                    nc.vector.tensor_tensor(
                        out=y1t[:, co, :], in0=y1[:, co, 0:510:2],
                        in1=y1[:, co, 1:510:2], op=OP.max)
                    nc.vector.tensor_tensor(
                        out=y1pA[:, co, g, :], in0=y1t[:, co, :],
                        in1=y1[:, co, 2:511:2], op=OP.max)
                for co in range(3):
                    nc.vector.tensor_tensor(
                        out=z1t[:, co, :], in0=z1[:, co, 0:510:2],
                        in1=z1[:, co, 1:510:2], op=OP.max)
                    nc.vector.tensor_tensor(
                        out=z1pA[:, co, g, :], in0=z1t[:, co, :],
                        in1=z1[:, co, 2:511:2], op=OP.max)

            # stage 2: batched over graph halves (2 graphs per matmul)
            y2 = ypool.tile([128, 2, GPC, L2Y], bf16, tag="y2")
            z2 = zpool.tile([128, 3, GPC, L2Z], bf16, tag="z2")
            for co in range(2):
                for gh in range(2):
                    ps = ps_hw.tile([128, 2, L2Y], f32, tag="pshw")
                    nc.tensor.matmul(
                        ps[:], c2w[:, co, :, :],
                        y1pA[:, 0:2, 2 * gh:2 * gh + 2, :],
                        start=True, stop=True, perf_mode=DR)
                    nc.scalar.activation(
                        y2[:, co, 2 * gh:2 * gh + 2, :], ps[:], AF.Relu,
                        scale=0.125,
                        bias=cbias[:, 2 + co:3 + co] if has_conv_b else 0.0)
            for co in range(3):
                for gh in range(2):
                    ps = ps_hw.tile([128, 2, L2Z], f32, tag="pshw")
                    for k in range(2):
                        nc.tensor.matmul(
                            ps[:], cc2w[:, 0:2, co, k, :],
                            z1pA[:, 0:2, 2 * gh:2 * gh + 2, k:k + L2Z],
                            start=(k == 0), stop=False, perf_mode=DR)
                    for k in range(2):
                        nc.tensor.matmul(
                            ps[:], cc2w[:, 2, co, k, :],
                            z1pA[:, 2, 2 * gh:2 * gh + 2, k:k + L2Z],
                            start=False, stop=(k == 1))
                    nc.scalar.activation(
                        z2[:, co, 2 * gh:2 * gh + 2, :], ps[:], AF.Relu,
                        scale=0.125,
                        bias=cbias[:, 7 + co:8 + co] if has_conv_b else 0.0)

            # pool2 + mlp heads, batched over all graphs
            y2p = ypool.tile([128, 2, GPC, P2], bf16, tag="y2p")
            z2p = zpool.tile([128, 3, GPC, P2], bf16, tag="z2p")
            for g in range(GPC):
                for co in range(2):
                    nc.vector.tensor_tensor(
                        out=y2p[:, co, g, :], in0=y2[:, co, g, 0:254:2],
                        in1=y2[:, co, g, 1:254:2], op=OP.max)
                for co in range(3):
                    nc.vector.tensor_tensor(
                        out=z2p[:, co, g, :], in0=z2[:, co, g, 0:254:2],
                        in1=z2[:, co, g, 1:254:2], op=OP.max)
            psy = ps_hw.tile([1, GPC, P2], f32, tag="pshw")
            mm_acc(nc, psy[:], [
                (mlpy[:, co:co + 1], y2p[:, co, :, :]) for co in range(2)])
            ys = ypool.tile([1, GPC, P2], f32, tag="ys")
            if has_mlp_b:
                nc.vector.tensor_scalar_add(
                    out=ys[:], in0=psy[:], scalar1=mbias[:, 0:1])
            else:
                nc.vector.tensor_copy(out=ys[:], in_=psy[:])
            psz = ps_hw.tile([1, GPC, P2], f32, tag="pshw")
            mm_acc(nc, psz[:], [
                (mlpz[:, co:co + 1], z2p[:, co, :, :]) for co in range(3)])
            prod = ypool.tile([1, GPC, P2], f32, tag="prod")
            if has_mlp_b:
                zs = zpool.tile([1, GPC, P2], f32, tag="zs")
                nc.vector.tensor_scalar_add(
                    out=zs[:], in0=psz[:], scalar1=mbias[:, 1:2])
                nc.vector.tensor_tensor(
                    out=prod[:], in0=ys[:], in1=zs[:], op=OP.mult)
            else:
                nc.vector.tensor_tensor(
                    out=prod[:], in0=ys[:], in1=psz[:], op=OP.mult)
            red = ypool.tile([1, GPC, 1], f32, tag="red")
            nc.vector.reduce_sum(red[:], prod[:], axis=mybir.AxisListType.X)
            nc.scalar.activation(
                out_sb[:1, :], red[:, :, 0], AF.Sigmoid, scale=1.0 / P2)

            nc.sync.dma_start(out=d_out[None, :], in_=out_sb[:1, :])

    nc.compile()
    return nc


def _layout_inputs(feature, W_msg, b_msg, gru_w_ih, gru_w_hh, gru_b_ih, gru_b_hh,
                   conv1_w, conv1_b, conv2_w, conv2_b, convc1_w, convc1_b,
                   convc2_w, convc2_b, mlpy_w, mlpy_b, mlpz_w, mlpz_b,
                   edge_src, edge_dst, edge_type):
    """Host-side sharding + SBUF-layout construction. Index math only
    (plus dtype casts / zero padding / transposes of float inputs)."""
    feature = np.asarray(feature, np.float32)
    edge_src = np.asarray(edge_src).astype(np.int64)
    edge_dst = np.asarray(edge_dst).astype(np.int64)
    edge_type = np.asarray(edge_type).astype(np.int64)

    flags = (
        bool(np.any(np.asarray(b_msg))),
        bool(np.any(np.asarray(gru_b_ih)) or np.any(np.asarray(gru_b_hh))),
        bool(np.any(np.asarray(conv1_b)) or np.any(np.asarray(conv2_b))
             or np.any(np.asarray(convc1_b)) or np.any(np.asarray(convc2_b))),
        bool(np.any(np.asarray(mlpy_b)) or np.any(np.asarray(mlpz_b))),
    )
    has_bmsg, has_gru_b, has_conv_b, has_mlp_b = flags

    # ---- adjacency counts ----
    g_of_e = edge_src // NN
    d_loc = edge_dst - g_of_e * NN
    # packed per-core row index of src: 513*(g mod GPC) + s_loc
    src_packed = edge_src - (g_of_e // GPC) * (GPC * NN)
    kc = src_packed // 128           # packed chunk 0..16 (per core)
    pp = src_packed - kc * 128
    kloc = kc - 4 * (g_of_e % GPC)   # strip chunk 0..4
    # A_h[p, core, g, kloc*T+t, d]
    A_h = np.zeros((128, NCORES, GPC, SC, NN), np.float32)
    np.add.at(A_h, (pp, g_of_e // GPC, g_of_e % GPC,
                    kloc * T + edge_type, d_loc), 1.0)

    # ---- shared weight layouts ----
    W_msg = np.asarray(W_msg, np.float32)          # [T, out, in]
    wmsg_kto = np.ascontiguousarray(
        W_msg.transpose(2, 0, 1).reshape(2, 128, T * OUT)
        .transpose(1, 0, 2))                        # [p, k, (t,o)]
    wmsg_l = wmsg_kto[:, 0, :].astype(BF16)         # step-0 (k=0 only)
    wmsg8_l = (wmsg_kto * 8.0).astype(F8)
    wih_km = np.ascontiguousarray(
        np.asarray(gru_w_ih, np.float32).T.reshape(2, 128, 3 * OUT)
        .transpose(1, 0, 2))                        # [p, k, m]
    whh_km = np.ascontiguousarray(
        np.asarray(gru_w_hh, np.float32).T.reshape(2, 128, 3 * OUT)
        .transpose(1, 0, 2))
    wih8_l = (wih_km * 8.0).astype(F8)
    whh8_l = (whh_km * 8.0).astype(F8)
    whh0_l = (whh_km[:, 0, :] * 8.0).astype(BF16)

    def conv_pairs(w, nci, nco):
        # w: [cout, cin, k] -> [p, k, co, ci, f]
        w = np.asarray(w, np.float32)
        k = w.shape[2]
        out = np.zeros((128, k, nco, nci, 128), np.float32)
        for kk in range(k):
            wt = w[:, :, kk].T                      # [cin, cout]
            for co in range(nco):
                for ci in range(nci):
                    out[:, kk, co, ci, :] = wt[ci * 128:(ci + 1) * 128,
                                               co * 128:(co + 1) * 128]
        return (out * 8.0).astype(F8)

    c1w_l = conv_pairs(conv1_w, 2, 2)               # [p, 3, 2co, 2ci, f]
    cc1w_l = conv_pairs(convc1_w, 3, 3)             # [p, 3, 3co, 3ci, f]
    c2w_l = conv_pairs(conv2_w, 2, 2)[:, 0]        # [p, 2co, 2ci, f]
    # cc2: [p, ci, co, k, f]
    cc2_t = conv_pairs(convc2_w, 3, 3)              # [p, 2k, 3co, 3ci, f]
    cc2w_l = np.ascontiguousarray(
        cc2_t.astype(np.float32).transpose(0, 3, 2, 1, 4)).astype(F8)
    mlpy_l = np.ascontiguousarray(
        np.asarray(mlpy_w, np.float32).reshape(2, 128).T).astype(BF16)
    mlpz_l = np.ascontiguousarray(
        np.asarray(mlpz_w, np.float32).reshape(3, 128).T).astype(BF16)

    shared = dict(wmsg=wmsg_l, wmsg8=wmsg8_l, wih8=wih8_l, whh8=whh8_l,
                  whh0=whh0_l, c1w=c1w_l, c2w=c2w_l, cc1w=cc1w_l,
                  cc2w=cc2w_l, mlpy=mlpy_l, mlpz=mlpz_l)
    if has_bmsg:
        shared["bmsg"] = np.asarray(b_msg, np.float32)
    if has_gru_b:
        gb = np.zeros((128, 12), np.float32)
        gb[:, 0:6] = 8.0 * np.asarray(gru_b_ih, np.float32).reshape(6, 128).T
        gb[:, 6:12] = 8.0 * np.asarray(gru_b_hh, np.float32).reshape(6, 128).T
        shared["gbias"] = gb
    if has_conv_b:
        cb = np.zeros((128, 10), np.float32)
        cb[:, 0:2] = np.asarray(conv1_b, np.float32).reshape(2, 128).T
        cb[:, 2:4] = np.asarray(conv2_b, np.float32).reshape(2, 128).T
        cb[:, 4:7] = np.asarray(convc1_b, np.float32).reshape(3, 128).T
        cb[:, 7:10] = np.asarray(convc2_b, np.float32).reshape(3, 128).T
        shared["cbias"] = cb
    if has_mlp_b:
        shared["mbias"] = np.array(
            [[float(np.asarray(mlpy_b).reshape(-1)[0]),
              float(np.asarray(mlpz_b).reshape(-1)[0])]], np.float32)

    in_maps = []
    for c in range(NCORES):
        g0 = c * GPC
        feat_l = np.zeros((128, NP), np.float32)
        rows = feature[g0 * NN:(g0 + GPC) * NN]                # [2052, 128]
        feat_l[:, :NV] = rows.T
        A_l = np.ascontiguousarray(A_h[:, c]).astype(F8)       # [128,4,20,513]
        m = dict(shared)
        m["feat"] = feat_l.astype(BF16)
        m["feat8"] = feat_l.astype(F8)
        m["adj"] = A_l
        if has_bmsg:
            ind = np.zeros((T, NP), np.float32)
            for g in range(GPC):
                ed_g = (g_of_e // GPC == c) & (g_of_e % GPC == g)
                np.add.at(ind, (edge_type[ed_g], g * NN + d_loc[ed_g]), 1.0)
            m["indeg"] = ind
        in_maps.append(m)
    return flags, in_maps


def kernel(**inputs):
    from concourse.bass_utils import run_bass_kernel_spmd

    flags, in_maps = _layout_inputs(**inputs)
    if flags not in _prog_cache:
        _prog_cache[flags] = _build_program(flags)
    nc = _prog_cache[flags]
    res = run_bass_kernel_spmd(nc, in_maps, core_ids=list(range(NCORES)))
    out = np.concatenate([np.asarray(res.results[c]["out"], np.float32)
                          for c in range(NCORES)])
    return out
